# revision 2
# baseline (speedup 1.0000x reference)
"""GIN (3-layer) message-passing kernel for 8 Trainium2 NeuronCores, v2.

Structure (3 device launches, one per GIN layer):
  layer(h) = relu((h + A h) @ W + b)   -- gather h rows directly (L2's table
  is the raw input x, so no separate dense-matmul launch), aggregate per dst
  window in PSUM, then apply W per window with W as the stationary matmul
  operand and bias fused into the ReLU on the scalar engine.

  Per core (1D dst-node partition, 12500 nodes = 98 windows of 128):
   - edge stream sorted by (window batch, src quadrant, window, src); per
     (window, quadrant) segment sized to the max count over cores so the
     SPMD instruction stream is shared; segments pack back-to-back (tiles
     that straddle a window boundary issue one S-matmul per window).
   - per 128-edge tile: gpsimd.dma_gather of h[src] rows (bf16, 256B) +
     one DVE-built S matrix (S[e,r] = w_e * (slot_e == r)) per covered
     window; TensorE accumulates either G^T S (layers 1-2, transposed
     aggregate so W stays stationary) or S^T G (layer 3).
   - self term: identity matmul with the core's own h tile per window.
   - layer 3 pools before the W3 projection (both linear): P = onehot^T agg
     accumulated across windows, then P @ W3pad + n_g b3 on device; host
     sums the 8 partial [G, CP] outputs.
"""

import numpy as np
import concourse.bass as bass
import concourse.mybir as mybir
import concourse.tile as tile
from concourse import bacc
from concourse.bass_utils import run_bass_kernel_spmd

try:
    import ml_dtypes
    BF16NP = ml_dtypes.bfloat16
except ImportError:  # pragma: no cover
    import jax.numpy as jnp
    BF16NP = jnp.bfloat16

F32 = mybir.dt.float32
BF16 = mybir.dt.bfloat16
I16 = mybir.dt.int16
AOT = mybir.AluOpType
ACT = mybir.ActivationFunctionType

NCORES = 8
WIN = 128
WB = 6             # windows per psum batch (1 PSUM bank each)
QN = 25000         # nodes per quadrant table
MAXCALL = 1024     # dma_gather HW limit per call
SCRATCH = 16384


class Cfg:
    def __init__(self, N=100000, E=1600000, F=128, C=40, G=64):
        self.N, self.E, self.F, self.C, self.G = N, E, F, C, G
        self.CP = 64
        self.NPC = N // NCORES
        self.NW = -(-self.NPC // WIN)
        self.NPAD = self.NW * WIN
        self.QPAD = -(-QN // 128) * 128


class Plan:
    """Shared-structure edge stream. All structure (segment offsets, tile
    window spans, call layout, stop flags) is identical across cores;
    only per-core data arrays (idx/slot/wgt) differ."""

    def __init__(self, cfg, src, dst, ew):
        self.cfg = cfg
        NPC, NW = cfg.NPC, cfg.NW
        core = dst // NPC
        dstl = dst - core * NPC
        w = dstl // WIN
        slot = dstl % WIN
        q = src // QN
        srcl = (src - q * QN).astype(np.int64)

        nb = -(-NW // WB)
        self.batches = [(b * WB, min((b + 1) * WB, NW)) for b in range(nb)]

        cnt = np.zeros((NCORES, NW, 4), np.int64)
        np.add.at(cnt, (core, w, q), 1)
        M = cnt.max(axis=0)                      # [NW, 4]
        assert (M >= 1).all()

        # stream layout: for each batch, for each q: window segments of
        # length M[w, q] packed back-to-back; run padded to 128.
        seg_off = np.zeros((NW, 4), np.int64)    # absolute stream offset
        self.runs = []                           # (b, q, off, padlen)
        cur = 0
        for b, (wlo, whi) in enumerate(self.batches):
            for qq in range(4):
                off = cur
                for ww in range(wlo, whi):
                    seg_off[ww, qq] = cur
                    cur += M[ww, qq]
                pad = -(-(cur - off) // 128) * 128
                self.runs.append((b, qq, off, pad))
                cur = off + pad
        self.NSLOT = cur
        self.NT = cur // 128

        # calls: chunks of <=MAXCALL within each run
        self.calls = []                          # (q, slot_off, nidx)
        self.batch_calls = [[] for _ in range(nb)]
        for (b, qq, off, pad) in self.runs:
            for rel in range(0, pad, MAXCALL):
                nidx = min(MAXCALL, pad - rel)
                self.batch_calls[b].append(len(self.calls))
                self.calls.append((qq, off + rel, nidx))
        # tile -> (call, k)
        self.tile_call = {}
        for ci, (qq, coff, nidx) in enumerate(self.calls):
            for k in range(nidx // 128):
                self.tile_call[coff // 128 + k] = (ci, k)

        # tile -> list of (window, scol); scol data = slot or -1
        self.tile_w = [[] for _ in range(self.NT)]
        scol_of = {}
        for (b, qq, off, pad) in self.runs:
            wlo, whi = self.batches[b]
            for ww in range(wlo, whi):
                s0 = seg_off[ww, qq]
                s1 = s0 + M[ww, qq]
                for t in range(s0 // 128, -(-s1 // 128)):
                    scol_of[(t, ww)] = len(scol_of)
                    self.tile_w[t].append((ww, scol_of[(t, ww)]))
        self.NS = len(scol_of)
        # stop flag: last (t, w) matmul per window in stream order
        last_mm = {}
        for t in range(self.NT):
            for (ww, j) in self.tile_w[t]:
                last_mm[ww] = (t, j)
        self.stop_at = {(t, j): ww for ww, (t, j) in last_mm.items()}
        self.batch_tiles = []
        for b in range(nb):
            ts = set()
            for ci in self.batch_calls[b]:
                qq, coff, nidx = self.calls[ci]
                ts.update(range(coff // 128, (coff + nidx) // 128))
            self.batch_tiles.append(sorted(ts))

        # per-core data arrays
        order = np.lexsort((srcl, q, w, core))
        g_of_edge = ((core * NW + w) * 4 + q)
        sorted_g = g_of_edge[order]
        starts = np.searchsorted(sorted_g, np.arange(NCORES * NW * 4))
        rank = np.arange(len(order)) - starts[sorted_g]
        rank = rank[np.argsort(order, kind="stable")]
        pos = seg_off[w, q] + rank               # stream slot per edge
        self.idx = np.zeros((NCORES, self.NSLOT), np.int16)
        self.wgt = np.zeros((NCORES, self.NSLOT), np.float32)
        self.slotv = np.full((NCORES, 128, self.NS), -1.0, np.float32)
        self.idx[core, pos] = srcl.astype(np.int16)
        self.wgt[core, pos] = ew.astype(np.float32)
        t_of = pos // 128
        p_of = pos % 128
        scol_arr = np.full((self.NT, NW), -1, np.int64)
        for (t, ww), j in scol_of.items():
            scol_arr[t, ww] = j
        j_of = scol_arr[t_of, w]
        assert (j_of >= 0).all()
        self.slotv[core, p_of, j_of] = slot.astype(np.float32)

    def idx_wrapped(self, c):
        a = self.idx[c].reshape(-1, 16).T
        return np.ascontiguousarray(np.tile(a, (8, 1)))

    def wgt_col(self, c):
        return np.ascontiguousarray(self.wgt[c].reshape(self.NT, 128).T)


def _iota(n, m):
    return np.tile(np.arange(m, dtype=np.float32), (n, 1))


def build_layer(cfg, plan, FO, relu, pool):
    F = cfg.F
    nc = bacc.Bacc("TRN2", target_bir_lowering=False, debug=False,
                   num_devices=NCORES, dynamic_dma_scratch_size=SCRATCH)
    hq = [nc.dram_tensor(f"hq{i}", [cfg.QPAD, F], BF16,
                         kind="ExternalInput").ap() for i in range(4)]
    own_d = nc.dram_tensor("own", [cfg.NPAD, F], BF16, kind="ExternalInput").ap()
    id_d = nc.dram_tensor("ident", [128, 128], BF16, kind="ExternalInput").ap()
    io_d = nc.dram_tensor("iota", [128, 128], F32, kind="ExternalInput").ap()
    ix_d = nc.dram_tensor("eidx", [128, plan.NT * 8], I16,
                          kind="ExternalInput").ap()
    sl_d = nc.dram_tensor("eslot", [128, plan.NS], F32,
                          kind="ExternalInput").ap()
    wg_d = nc.dram_tensor("ewgt", [128, plan.NT], F32,
                          kind="ExternalInput").ap()
    if pool:
        ig_d = nc.dram_tensor("iotaG", [128, cfg.G], F32,
                              kind="ExternalInput").ap()
        bv_d = nc.dram_tensor("bvals", [128, cfg.NW], F32,
                              kind="ExternalInput").ap()
        idf_d = nc.dram_tensor("identf", [128, 128], F32,
                               kind="ExternalInput").ap()
        w3_d = nc.dram_tensor("W3p", [F, cfg.CP], BF16,
                              kind="ExternalInput").ap()
        ng_d = nc.dram_tensor("ngrow", [1, cfg.G], BF16,
                              kind="ExternalInput").ap()
        b3_d = nc.dram_tensor("b3row", [1, cfg.CP], BF16,
                              kind="ExternalInput").ap()
        out_d = nc.dram_tensor("pool", [cfg.G, cfg.CP], F32,
                               kind="ExternalOutput").ap()
    else:
        w_d = nc.dram_tensor("W", [F, FO], BF16, kind="ExternalInput").ap()
        b_d = nc.dram_tensor("bcol", [128, 1], F32, kind="ExternalInput").ap()
        out_d = nc.dram_tensor("hT", [FO, cfg.NPAD], BF16,
                               kind="ExternalOutput").ap()

    with tile.TileContext(nc) as tc:
        with tc.tile_pool(name="cst", bufs=1) as cst, \
             tc.tile_pool(name="meta", bufs=1) as meta, \
             tc.tile_pool(name="gath", bufs=6) as gath, \
             tc.tile_pool(name="sp", bufs=10) as sp, \
             tc.tile_pool(name="io", bufs=8) as io, \
             tc.tile_pool(name="aps", bufs=WB, space="PSUM") as aps, \
             tc.tile_pool(name="ops", bufs=1 if pool else 2,
                          space="PSUM") as ops:
            id_sb = cst.tile([128, 128], BF16)
            nc.sync.dma_start(out=id_sb[:], in_=id_d[:])
            iota_sb = cst.tile([128, 128], F32)
            nc.sync.dma_start(out=iota_sb[:], in_=io_d[:])
            if pool:
                ig_sb = cst.tile([128, cfg.G], F32)
                nc.sync.dma_start(out=ig_sb[:], in_=ig_d[:])
                bv_sb = cst.tile([128, cfg.NW], F32)
                nc.sync.dma_start(out=bv_sb[:], in_=bv_d[:])
                idf_sb = cst.tile([128, 128], F32)
                nc.sync.dma_start(out=idf_sb[:], in_=idf_d[:])
                w3_sb = cst.tile([F, cfg.CP], BF16)
                nc.sync.dma_start(out=w3_sb[:], in_=w3_d[:])
                ng_sb = cst.tile([1, cfg.G], BF16)
                nc.sync.dma_start(out=ng_sb[:], in_=ng_d[:])
                b3_sb = cst.tile([1, cfg.CP], BF16)
                nc.sync.dma_start(out=b3_sb[:], in_=b3_d[:])
                pool_ps = ops.tile([cfg.G, F], F32, tag="pool", name="pool_ps")
            else:
                w_sb = cst.tile([F, FO], BF16)
                nc.sync.dma_start(out=w_sb[:], in_=w_d[:])
                b_sb = cst.tile([128, 1], F32)
                nc.sync.dma_start(out=b_sb[:], in_=b_d[:])
            ix_sb = meta.tile([128, plan.NT * 8], I16)
            nc.sync.dma_start(out=ix_sb[:], in_=ix_d[:])
            sl_sb = meta.tile([128, plan.NS], F32)
            nc.sync.dma_start(out=sl_sb[:], in_=sl_d[:])
            wg_sb = meta.tile([128, plan.NT], F32)
            nc.sync.dma_start(out=wg_sb[:], in_=wg_d[:])
            own_sb = meta.tile([128, cfg.NW, F], BF16)
            own_r = own_d.rearrange("(n p) d -> p n d", p=128)
            nc.sync.dma_start(out=own_sb[:], in_=own_r[:])

            for b, (wlo, whi) in enumerate(plan.batches):
                tile_src = {}
                for ci in plan.batch_calls[b]:
                    qq, coff, nidx = plan.calls[ci]
                    ct = nidx // 128
                    gt = gath.tile([128, ct, F], BF16, tag="gt")
                    nc.gpsimd.dma_gather(
                        gt[:], hq[qq][:], ix_sb[:, coff // 16:(coff + nidx) // 16],
                        nidx, nidx, F)
                    for k in range(ct):
                        tile_src[coff // 128 + k] = (gt, k)
                wps = {}
                for ww in range(wlo, whi):
                    pt = aps.tile([128, WIN], F32, tag="agg",
                                  name=f"agg_{b}_{ww}")
                    wps[ww] = pt[:]
                for ww in range(wlo, whi):
                    pw = wps[ww]
                    if pool:
                        nc.tensor.matmul(out=pw, lhsT=id_sb[:],
                                         rhs=own_sb[:, ww, :],
                                         start=True, stop=False)
                    else:
                        nc.tensor.matmul(out=pw, lhsT=own_sb[:, ww, :],
                                         rhs=id_sb[:], start=True, stop=False)
                for t in plan.batch_tiles[b]:
                    gt, k = tile_src[t]
                    for (ww, j) in plan.tile_w[t]:
                        s_t = sp.tile([128, WIN], BF16, tag="S")
                        nc.vector.tensor_scalar(
                            out=s_t[:], in0=iota_sb[:],
                            scalar1=sl_sb[:, j:j + 1],
                            scalar2=wg_sb[:, t:t + 1],
                            op0=AOT.is_equal, op1=AOT.mult)
                        is_stop = plan.stop_at.get((t, j)) == ww
                        if pool:
                            nc.tensor.matmul(out=wps[ww], lhsT=s_t[:],
                                             rhs=gt[:, k, :],
                                             start=False, stop=is_stop)
                        else:
                            nc.tensor.matmul(out=wps[ww], lhsT=gt[:, k, :],
                                             rhs=s_t[:],
                                             start=False, stop=is_stop)
                for ww in range(wlo, whi):
                    pw = wps[ww]
                    if pool:
                        agg_sb = io.tile([128, F], BF16, tag="agg_sb")
                        nc.vector.tensor_copy(out=agg_sb[:], in_=pw)
                        s_p = sp.tile([128, cfg.G], BF16, tag="SP")
                        nc.vector.tensor_scalar(
                            out=s_p[:], in0=ig_sb[:],
                            scalar1=bv_sb[:, ww:ww + 1], scalar2=None,
                            op0=AOT.is_equal)
                        nc.tensor.matmul(out=pool_ps[:], lhsT=s_p[:],
                                         rhs=agg_sb[:], start=(ww == 0),
                                         stop=(ww == cfg.NW - 1))
                    else:
                        aggT = io.tile([F, 128], BF16, tag="aggT")
                        nc.vector.tensor_copy(out=aggT[:], in_=pw)
                        preT = ops.tile([FO, 128], F32, tag="preT")
                        nc.tensor.matmul(out=preT[:], lhsT=w_sb[:],
                                         rhs=aggT[:], start=True, stop=True)
                        hT = io.tile([FO, 128], BF16, tag="hT")
                        nc.scalar.activation(
                            out=hT[:], in_=preT[:],
                            func=ACT.Relu if relu else ACT.Copy,
                            bias=b_sb[:])
                        nc.sync.dma_start(
                            out=out_d[:, ww * 128:(ww + 1) * 128], in_=hT[:])
            if pool:
                pool_sb = io.tile([cfg.G, F], F32, tag="pool_sb")
                nc.vector.tensor_copy(out=pool_sb[:], in_=pool_ps[:])
                ptT_ps = ops.tile([F, cfg.G], F32, tag="pool")
                nc.tensor.transpose(out=ptT_ps[:], in_=pool_sb[:],
                                    identity=idf_sb[0:cfg.G, 0:cfg.G])
                ptT = io.tile([F, cfg.G], BF16, tag="ptTsb")
                nc.vector.tensor_copy(out=ptT[:], in_=ptT_ps[:])
                out_ps = ops.tile([cfg.G, cfg.CP], F32, tag="pool")
                nc.tensor.matmul(out=out_ps[:], lhsT=ptT[:], rhs=w3_sb[:],
                                 start=True, stop=False)
                nc.tensor.matmul(out=out_ps[:], lhsT=ng_sb[:], rhs=b3_sb[:],
                                 start=False, stop=True)
                out_sb = io.tile([cfg.G, cfg.CP], F32, tag="out_sb")
                nc.vector.tensor_copy(out=out_sb[:], in_=out_ps[:])
                nc.sync.dma_start(out=out_d[:], in_=out_sb[:])
    nc.compile()
    return nc


def _bf16(a):
    return np.ascontiguousarray(np.asarray(a, dtype=np.float32)).astype(BF16NP)


def _tables(cfg, h):
    """4 contiguous quadrant tables (bf16, padded) + per-core own slabs."""
    out = []
    for i in range(4):
        t = np.zeros((cfg.QPAD, cfg.F), np.float32)
        t[:QN] = h[i * QN:(i + 1) * QN]
        out.append(_bf16(t))
    owns = []
    for c in range(NCORES):
        o = np.zeros((cfg.NPAD, cfg.F), np.float32)
        o[:cfg.NPC] = h[c * cfg.NPC:(c + 1) * cfg.NPC]
        owns.append(_bf16(o))
    return out, owns


def _layer_inputs(cfg, plan, tables, owns):
    maps = []
    ident = _bf16(np.eye(128, dtype=np.float32))
    iota = _iota(128, 128)
    for c in range(NCORES):
        m = {f"hq{i}": tables[i] for i in range(4)}
        m["own"] = owns[c]
        m["ident"] = ident
        m["iota"] = iota
        m["eidx"] = plan.idx_wrapped(c)
        m["eslot"] = np.ascontiguousarray(plan.slotv[c])
        m["ewgt"] = plan.wgt_col(c)
        maps.append(m)
    return maps


def _run(nc, in_maps):
    return run_bass_kernel_spmd(nc, in_maps, core_ids=list(range(NCORES))).results


def gin_forward(cfg, x, edge_index, edge_weight, batch,
                W1, b1, W2, b2, W3, b3, ncs=None):
    src = np.asarray(edge_index[0], np.int64)
    dst = np.asarray(edge_index[1], np.int64)
    plan = Plan(cfg, src, dst, np.asarray(edge_weight, np.float32))
    if ncs is None:
        ncs = {}
    if "l2" not in ncs:
        ncs["l2"] = build_layer(cfg, plan, cfg.F, True, False)
        ncs["l3"] = build_layer(cfg, plan, cfg.F, True, False)
        ncs["l4"] = build_layer(cfg, plan, None, False, True)

    # layer 1: tables = x
    h = np.asarray(x, np.float32)
    tables, owns = _tables(cfg, h)
    maps = _layer_inputs(cfg, plan, tables, owns)
    for m in maps:
        m["W"] = _bf16(W1)
        m["bcol"] = np.asarray(b1, np.float32).reshape(128, 1)
    res = _run(ncs["l2"], maps)
    h1 = np.concatenate(
        [np.asarray(res[c]["hT"], np.float32).T[:cfg.NPC] for c in range(NCORES)])

    tables, owns = _tables(cfg, h1)
    maps = _layer_inputs(cfg, plan, tables, owns)
    for m in maps:
        m["W"] = _bf16(W2)
        m["bcol"] = np.asarray(b2, np.float32).reshape(128, 1)
    res = _run(ncs["l3"], maps)
    h2 = np.concatenate(
        [np.asarray(res[c]["hT"], np.float32).T[:cfg.NPC] for c in range(NCORES)])

    # layer 3 + pool
    W3p = np.zeros((cfg.F, cfg.CP), np.float32)
    W3p[:, :cfg.C] = np.asarray(W3, np.float32)
    b3p = np.zeros(cfg.CP, np.float32)
    b3p[:cfg.C] = np.asarray(b3, np.float32)
    batch64 = np.asarray(batch, np.int64)
    ng = np.bincount(batch64, minlength=cfg.G).astype(np.float32)
    tables, owns = _tables(cfg, h2)
    maps = _layer_inputs(cfg, plan, tables, owns)
    for c, m in enumerate(maps):
        m["iotaG"] = _iota(128, cfg.G)
        bv = np.full(cfg.NPAD, -1.0, np.float32)
        bv[:cfg.NPC] = batch64[c * cfg.NPC:(c + 1) * cfg.NPC].astype(np.float32)
        m["bvals"] = np.ascontiguousarray(bv.reshape(cfg.NW, 128).T)
        m["identf"] = np.eye(128, dtype=np.float32)
        m["W3p"] = _bf16(W3p)
        # bias counted once globally: fold n_g b3 into core 0's partial only
        m["ngrow"] = _bf16((ng if c == 0 else np.zeros_like(ng)).reshape(1, -1))
        m["b3row"] = _bf16(b3p.reshape(1, -1))
    res = _run(ncs["l4"], maps)
    out = np.zeros((cfg.G, cfg.CP), np.float32)
    for c in range(NCORES):
        out += np.asarray(res[c]["pool"], np.float32)
    return out[:, :cfg.C].astype(np.float32)


def kernel(x, edge_index, edge_weight, batch, W1, b1, W2, b2, W3, b3):
    cfg = Cfg()
    return gin_forward(cfg, x, edge_index, edge_weight, batch,
                       W1, b1, W2, b2, W3, b3)


# revision 4
# speedup vs baseline: 2.2914x; 2.2914x over previous
"""GIN (3-layer) message-passing kernel for 8 Trainium2 NeuronCores, v2.

Structure (3 device launches, one per GIN layer):
  layer(h) = relu((h + A h) @ W + b)   -- gather h rows directly (L2's table
  is the raw input x, so no separate dense-matmul launch), aggregate per dst
  window in PSUM, then apply W per window with W as the stationary matmul
  operand and bias fused into the ReLU on the scalar engine.

  Per core (1D dst-node partition, 12500 nodes = 98 windows of 128):
   - edge stream sorted by (window batch, src quadrant, window, src); per
     (window, quadrant) segment sized to the max count over cores so the
     SPMD instruction stream is shared; segments pack back-to-back (tiles
     that straddle a window boundary issue one S-matmul per window).
   - per 128-edge tile: gpsimd.dma_gather of h[src] rows (bf16, 256B) +
     one DVE-built S matrix (S[e,r] = w_e * (slot_e == r)) per covered
     window; TensorE accumulates either G^T S (layers 1-2, transposed
     aggregate so W stays stationary) or S^T G (layer 3).
   - self term: identity matmul with the core's own h tile per window.
   - layer 3 pools before the W3 projection (both linear): P = onehot^T agg
     accumulated across windows, then P @ W3pad + n_g b3 on device; host
     sums the 8 partial [G, CP] outputs.
"""

import numpy as np
import concourse.bass as bass
import concourse.mybir as mybir
import concourse.tile as tile
from concourse import bacc
from concourse.bass_utils import run_bass_kernel_spmd

try:
    import ml_dtypes
    BF16NP = ml_dtypes.bfloat16
except ImportError:  # pragma: no cover
    import jax.numpy as jnp
    BF16NP = jnp.bfloat16

F32 = mybir.dt.float32
BF16 = mybir.dt.bfloat16
I16 = mybir.dt.int16
AOT = mybir.AluOpType
ACT = mybir.ActivationFunctionType

NCORES = 8
WIN = 128
WB = 6             # windows per psum batch (1 PSUM bank each)
QN = 25000         # nodes per quadrant table
MAXCALL = 1024     # dma_gather HW limit per call
SCRATCH = 16384


class Cfg:
    def __init__(self, N=100000, E=1600000, F=128, C=40, G=64):
        self.N, self.E, self.F, self.C, self.G = N, E, F, C, G
        self.CP = 64
        self.NPC = N // NCORES
        self.NW = -(-self.NPC // WIN)
        self.NPAD = self.NW * WIN
        self.QPAD = -(-QN // 128) * 128


class Plan:
    """Shared-structure edge stream. All structure (segment offsets, tile
    window spans, call layout, stop flags) is identical across cores;
    only per-core data arrays (idx/slot/wgt) differ."""

    def __init__(self, cfg, src, dst, ew):
        self.cfg = cfg
        NPC, NW = cfg.NPC, cfg.NW
        core = dst // NPC
        dstl = dst - core * NPC
        w = dstl // WIN
        slot = dstl % WIN
        q = src // QN
        srcl = (src - q * QN).astype(np.int64)

        nb = -(-NW // WB)
        self.batches = [(b * WB, min((b + 1) * WB, NW)) for b in range(nb)]

        cnt = np.zeros((NCORES, NW, 4), np.int64)
        np.add.at(cnt, (core, w, q), 1)
        assert (cnt.max(axis=0) >= 1).all()
        # segment length = ceil(mean over cores); per-core overflow edges go
        # to a shared overflow zone at the end of each (batch, q) run.
        L = np.ceil(cnt.mean(axis=0)).astype(np.int64)   # [NW, 4]
        ov = np.maximum(cnt - L[None], 0)                # [C, NW, 4]

        seg_off = np.zeros((NW, 4), np.int64)    # absolute stream offset
        runbase = np.zeros((NW, 4), np.int64)    # overflow zone start of run
        self.runs = []                           # (b, q, off, padlen, seglen)
        cur = 0
        for b, (wlo, whi) in enumerate(self.batches):
            for qq in range(4):
                off = cur
                for ww in range(wlo, whi):
                    seg_off[ww, qq] = cur
                    cur += L[ww, qq]
                seglen = cur - off
                ov_max = int(ov[:, wlo:whi, qq].sum(axis=1).max())
                pad = -(-(seglen + ov_max) // 128) * 128
                runbase[wlo:whi, qq] = off + seglen
                self.runs.append((b, qq, off, pad, seglen))
                cur = off + pad
        self.NSLOT = cur
        self.NT = cur // 128

        # calls: chunks of <=MAXCALL within each run
        self.calls = []                          # (q, slot_off, nidx)
        self.batch_calls = [[] for _ in range(nb)]
        for (b, qq, off, pad, seglen) in self.runs:
            for rel in range(0, pad, MAXCALL):
                nidx = min(MAXCALL, pad - rel)
                self.batch_calls[b].append(len(self.calls))
                self.calls.append((qq, off + rel, nidx))
        # tile -> (call, k)
        self.tile_call = {}
        for ci, (qq, coff, nidx) in enumerate(self.calls):
            for k in range(nidx // 128):
                self.tile_call[coff // 128 + k] = (ci, k)

        # windows covered per tile: window segments, plus every batch
        # window for tiles overlapping the run's overflow zone.
        wsets = [set() for _ in range(self.NT)]
        for (b, qq, off, pad, seglen) in self.runs:
            wlo, whi = self.batches[b]
            for ww in range(wlo, whi):
                s0 = seg_off[ww, qq]
                s1 = s0 + L[ww, qq]
                for t in range(s0 // 128, -(-s1 // 128)):
                    wsets[t].add(ww)
            for t in range((off + seglen) // 128, (off + pad) // 128):
                wsets[t].update(range(wlo, whi))
        self.batch_tiles = []
        for b in range(nb):
            ts = set()
            for ci in self.batch_calls[b]:
                qq, coff, nidx = self.calls[ci]
                ts.update(range(coff // 128, (coff + nidx) // 128))
            self.batch_tiles.append(sorted(ts))
        # assign scol ids in execution order (batch-major, tile, window)
        # so each batch's S columns are one contiguous block in HBM.
        self.tile_w = [[] for _ in range(self.NT)]
        scol_of = {}
        self.batch_scol = []
        for b in range(nb):
            j_lo = len(scol_of)
            for t in self.batch_tiles[b]:
                for ww in sorted(wsets[t]):
                    scol_of[(t, ww)] = len(scol_of)
                    self.tile_w[t].append((ww, scol_of[(t, ww)]))
            self.batch_scol.append((j_lo, len(scol_of)))
        self.NS = len(scol_of)
        self.t_of_scol = np.zeros(self.NS, np.int64)
        for (t, ww), j in scol_of.items():
            self.t_of_scol[j] = t
        # stop flag: last (t, w) matmul per window in stream order
        last_mm = {}
        for t in range(self.NT):
            for (ww, j) in self.tile_w[t]:
                last_mm[ww] = (t, j)
        self.stop_at = {(t, j): ww for ww, (t, j) in last_mm.items()}

        # per-core data arrays
        order = np.lexsort((srcl, q, w, core))
        g_of_edge = ((core * NW + w) * 4 + q)
        sorted_g = g_of_edge[order]
        starts = np.searchsorted(sorted_g, np.arange(NCORES * NW * 4))
        rank = np.arange(len(order)) - starts[sorted_g]
        rank = rank[np.argsort(order, kind="stable")]
        # placement: first L edges of each (core, w, q) cell go to the
        # window segment; the rest go to the run's overflow zone, packed
        # in window order per core.
        opx = np.cumsum(ov, axis=1) - ov         # exclusive prefix within NW
        for b, (wlo, whi) in enumerate(self.batches):
            opx[:, wlo:whi, :] -= opx[:, wlo:wlo + 1, :]
        inseg = rank < L[w, q]
        pos = np.where(inseg, seg_off[w, q] + rank,
                       runbase[w, q] + opx[core, w, q] + (rank - L[w, q]))
        self.idx = np.zeros((NCORES, self.NSLOT), np.int16)
        self.wgt = np.zeros((NCORES, self.NSLOT), np.float32)
        self.slotv = np.full((NCORES, 128, self.NS), -1.0, np.float32)
        self.idx[core, pos] = srcl.astype(np.int16)
        self.wgt[core, pos] = ew.astype(np.float32)
        t_of = pos // 128
        p_of = pos % 128
        scol_arr = np.full((self.NT, NW), -1, np.int64)
        for (t, ww), j in scol_of.items():
            scol_arr[t, ww] = j
        j_of = scol_arr[t_of, w]
        assert (j_of >= 0).all()
        self.slotv[core, p_of, j_of] = slot.astype(np.float32)

    def idx_wrapped(self, c):
        a = self.idx[c].reshape(-1, 16).T
        return np.ascontiguousarray(np.tile(a, (8, 1)))

    def smat(self, c):
        """Precomputed S tiles [128, NS, 128] bf16: S[p, j, r] =
        wgt[t(j)*128+p] * (slotv[p, j] == r)."""
        if not hasattr(self, "_smat"):
            self._smat = {}
        if c not in self._smat:
            out = np.zeros((128, self.NS, 128), BF16NP)
            wt = self.wgt[c].reshape(self.NT, 128)
            r = np.arange(128)
            for j0 in range(0, self.NS, 256):
                j1 = min(j0 + 256, self.NS)
                sl = self.slotv[c][:, j0:j1]
                blk = (sl[:, :, None] == r).astype(np.float32)
                blk *= wt[self.t_of_scol[j0:j1]].T[:, :, None]
                out[:, j0:j1, :] = blk.astype(BF16NP)
            self._smat[c] = out
        return self._smat[c]


def _iota(n, m):
    return np.tile(np.arange(m, dtype=np.float32), (n, 1))


def build_layer(cfg, plan, FO, relu, pool):
    F = cfg.F
    nc = bacc.Bacc("TRN2", target_bir_lowering=False, debug=False,
                   num_devices=NCORES, dynamic_dma_scratch_size=SCRATCH)
    hq = [nc.dram_tensor(f"hq{i}", [cfg.QPAD, F], BF16,
                         kind="ExternalInput").ap() for i in range(4)]
    own_d = nc.dram_tensor("own", [cfg.NPAD, F], BF16, kind="ExternalInput").ap()
    id_d = nc.dram_tensor("ident", [128, 128], BF16, kind="ExternalInput").ap()
    ix_d = nc.dram_tensor("eidx", [128, plan.NT * 8], I16,
                          kind="ExternalInput").ap()
    s_d = nc.dram_tensor("smat", [128, plan.NS, WIN], BF16,
                         kind="ExternalInput").ap()
    if pool:
        ig_d = nc.dram_tensor("iotaG", [128, cfg.G], F32,
                              kind="ExternalInput").ap()
        bv_d = nc.dram_tensor("bvals", [128, cfg.NW], F32,
                              kind="ExternalInput").ap()
        idf_d = nc.dram_tensor("identf", [128, 128], F32,
                               kind="ExternalInput").ap()
        w3_d = nc.dram_tensor("W3p", [F, cfg.CP], BF16,
                              kind="ExternalInput").ap()
        ng_d = nc.dram_tensor("ngrow", [1, cfg.G], BF16,
                              kind="ExternalInput").ap()
        b3_d = nc.dram_tensor("b3row", [1, cfg.CP], BF16,
                              kind="ExternalInput").ap()
        out_d = nc.dram_tensor("pool", [cfg.G, cfg.CP], F32,
                               kind="ExternalOutput").ap()
    else:
        w_d = nc.dram_tensor("W", [F, FO], BF16, kind="ExternalInput").ap()
        b_d = nc.dram_tensor("bcol", [128, 1], F32, kind="ExternalInput").ap()
        out_d = nc.dram_tensor("hT", [FO, cfg.NPAD], BF16,
                               kind="ExternalOutput").ap()

    with tile.TileContext(nc) as tc:
        with tc.tile_pool(name="cst", bufs=1) as cst, \
             tc.tile_pool(name="meta", bufs=1) as meta, \
             tc.tile_pool(name="gath", bufs=6) as gath, \
             tc.tile_pool(name="smat", bufs=2) as spool, \
             tc.tile_pool(name="sp", bufs=10) as sp, \
             tc.tile_pool(name="io", bufs=8) as io, \
             tc.tile_pool(name="aps", bufs=WB, space="PSUM") as aps, \
             tc.tile_pool(name="ops", bufs=1 if pool else 2,
                          space="PSUM") as ops:
            id_sb = cst.tile([128, 128], BF16)
            nc.sync.dma_start(out=id_sb[:], in_=id_d[:])
            if pool:
                ig_sb = cst.tile([128, cfg.G], F32)
                nc.sync.dma_start(out=ig_sb[:], in_=ig_d[:])
                bv_sb = cst.tile([128, cfg.NW], F32)
                nc.sync.dma_start(out=bv_sb[:], in_=bv_d[:])
                idf_sb = cst.tile([128, 128], F32)
                nc.sync.dma_start(out=idf_sb[:], in_=idf_d[:])
                w3_sb = cst.tile([F, cfg.CP], BF16)
                nc.sync.dma_start(out=w3_sb[:], in_=w3_d[:])
                ng_sb = cst.tile([1, cfg.G], BF16)
                nc.sync.dma_start(out=ng_sb[:], in_=ng_d[:])
                b3_sb = cst.tile([1, cfg.CP], BF16)
                nc.sync.dma_start(out=b3_sb[:], in_=b3_d[:])
                pool_ps = ops.tile([cfg.G, F], F32, tag="pool", name="pool_ps")
            else:
                w_sb = cst.tile([F, FO], BF16)
                nc.sync.dma_start(out=w_sb[:], in_=w_d[:])
                b_sb = cst.tile([128, 1], F32)
                nc.sync.dma_start(out=b_sb[:], in_=b_d[:])
            ix_sb = meta.tile([128, plan.NT * 8], I16)
            # batch 0's index columns first so the first gather starts early
            qq0, coff0, nidx0 = plan.calls[plan.batch_calls[0][-1]]
            c_split = (coff0 + nidx0) // 16
            nc.sync.dma_start(out=ix_sb[:, :c_split], in_=ix_d[:, :c_split])
            nc.sync.dma_start(out=ix_sb[:, c_split:], in_=ix_d[:, c_split:])
            own_sb = meta.tile([128, cfg.NW, F], BF16)
            own_r = own_d.rearrange("(n p) d -> p n d", p=128)
            nc.sync.dma_start(out=own_sb[:], in_=own_r[:])

            for b, (wlo, whi) in enumerate(plan.batches):
                tile_src = {}
                for ci in plan.batch_calls[b]:
                    qq, coff, nidx = plan.calls[ci]
                    ct = nidx // 128
                    gt = gath.tile([128, ct, F], BF16, tag="gt")
                    nc.gpsimd.dma_gather(
                        gt[:], hq[qq][:], ix_sb[:, coff // 16:(coff + nidx) // 16],
                        nidx, nidx, F)
                    for k in range(ct):
                        tile_src[coff // 128 + k] = (gt, k)
                j_lo, j_hi = plan.batch_scol[b]
                sblk = spool.tile([128, j_hi - j_lo, WIN], BF16, tag="sblk")
                nc.sync.dma_start(out=sblk[:], in_=s_d[:, j_lo:j_hi, :])
                wps = {}
                for ww in range(wlo, whi):
                    pt = aps.tile([128, WIN], F32, tag="agg",
                                  name=f"agg_{b}_{ww}")
                    wps[ww] = pt[:]
                for ww in range(wlo, whi):
                    pw = wps[ww]
                    if pool:
                        nc.tensor.matmul(out=pw, lhsT=id_sb[:],
                                         rhs=own_sb[:, ww, :],
                                         start=True, stop=False)
                    else:
                        nc.tensor.matmul(out=pw, lhsT=own_sb[:, ww, :],
                                         rhs=id_sb[:], start=True, stop=False)
                for t in plan.batch_tiles[b]:
                    gt, k = tile_src[t]
                    for (ww, j) in plan.tile_w[t]:
                        s_t = sblk[:, j - j_lo, :]
                        is_stop = plan.stop_at.get((t, j)) == ww
                        if pool:
                            nc.tensor.matmul(out=wps[ww], lhsT=s_t,
                                             rhs=gt[:, k, :],
                                             start=False, stop=is_stop)
                        else:
                            nc.tensor.matmul(out=wps[ww], lhsT=gt[:, k, :],
                                             rhs=s_t,
                                             start=False, stop=is_stop)
                for ww in range(wlo, whi):
                    pw = wps[ww]
                    if pool:
                        agg_sb = io.tile([128, F], BF16, tag="agg_sb")
                        nc.vector.tensor_copy(out=agg_sb[:], in_=pw)
                        s_p = sp.tile([128, cfg.G], BF16, tag="SP")
                        nc.vector.tensor_scalar(
                            out=s_p[:], in0=ig_sb[:],
                            scalar1=bv_sb[:, ww:ww + 1], scalar2=None,
                            op0=AOT.is_equal)
                        nc.tensor.matmul(out=pool_ps[:], lhsT=s_p[:],
                                         rhs=agg_sb[:], start=(ww == 0),
                                         stop=(ww == cfg.NW - 1))
                    else:
                        aggT = io.tile([F, 128], BF16, tag="aggT")
                        nc.vector.tensor_copy(out=aggT[:], in_=pw)
                        preT = ops.tile([FO, 128], F32, tag="preT")
                        nc.tensor.matmul(out=preT[:], lhsT=w_sb[:],
                                         rhs=aggT[:], start=True, stop=True)
                        hT = io.tile([FO, 128], BF16, tag="hT")
                        nc.scalar.activation(
                            out=hT[:], in_=preT[:],
                            func=ACT.Relu if relu else ACT.Copy,
                            bias=b_sb[:])
                        nc.sync.dma_start(
                            out=out_d[:, ww * 128:(ww + 1) * 128], in_=hT[:])
            if pool:
                pool_sb = io.tile([cfg.G, F], F32, tag="pool_sb")
                nc.vector.tensor_copy(out=pool_sb[:], in_=pool_ps[:])
                ptT_ps = ops.tile([F, cfg.G], F32, tag="pool")
                nc.tensor.transpose(out=ptT_ps[:], in_=pool_sb[:],
                                    identity=idf_sb[0:cfg.G, 0:cfg.G])
                ptT = io.tile([F, cfg.G], BF16, tag="ptTsb")
                nc.vector.tensor_copy(out=ptT[:], in_=ptT_ps[:])
                out_ps = ops.tile([cfg.G, cfg.CP], F32, tag="pool")
                nc.tensor.matmul(out=out_ps[:], lhsT=ptT[:], rhs=w3_sb[:],
                                 start=True, stop=False)
                nc.tensor.matmul(out=out_ps[:], lhsT=ng_sb[:], rhs=b3_sb[:],
                                 start=False, stop=True)
                out_sb = io.tile([cfg.G, cfg.CP], F32, tag="out_sb")
                nc.vector.tensor_copy(out=out_sb[:], in_=out_ps[:])
                nc.sync.dma_start(out=out_d[:], in_=out_sb[:])
    nc.compile()
    return nc


def build_pool(cfg):
    """Final layer: P = C^T @ h2_own over the core's own rows (C is the
    host-precomputed pool-of-(I+A) operator restricted to this core's
    columns), then out = P @ W3pad + n_g b3. No gathers."""
    F = cfg.F
    nc = bacc.Bacc("TRN2", target_bir_lowering=False, debug=False,
                   num_devices=NCORES, dynamic_dma_scratch_size=SCRATCH)
    own_d = nc.dram_tensor("own", [cfg.NPAD, F], BF16, kind="ExternalInput").ap()
    c_d = nc.dram_tensor("Cmat", [cfg.NPAD, cfg.G], BF16,
                         kind="ExternalInput").ap()
    idf_d = nc.dram_tensor("identf", [128, 128], F32, kind="ExternalInput").ap()
    w3_d = nc.dram_tensor("W3p", [F, cfg.CP], BF16, kind="ExternalInput").ap()
    ng_d = nc.dram_tensor("ngrow", [1, cfg.G], BF16, kind="ExternalInput").ap()
    b3_d = nc.dram_tensor("b3row", [1, cfg.CP], BF16, kind="ExternalInput").ap()
    out_d = nc.dram_tensor("pool", [cfg.G, cfg.CP], F32,
                           kind="ExternalOutput").ap()
    with tile.TileContext(nc) as tc:
        with tc.tile_pool(name="cst", bufs=1) as cst, \
             tc.tile_pool(name="io", bufs=4) as io, \
             tc.tile_pool(name="ops", bufs=1, space="PSUM") as ops:
            idf_sb = cst.tile([128, 128], F32)
            nc.sync.dma_start(out=idf_sb[:], in_=idf_d[:])
            w3_sb = cst.tile([F, cfg.CP], BF16)
            nc.sync.dma_start(out=w3_sb[:], in_=w3_d[:])
            ng_sb = cst.tile([1, cfg.G], BF16)
            nc.sync.dma_start(out=ng_sb[:], in_=ng_d[:])
            b3_sb = cst.tile([1, cfg.CP], BF16)
            nc.sync.dma_start(out=b3_sb[:], in_=b3_d[:])
            own_sb = cst.tile([128, cfg.NW, F], BF16)
            nc.sync.dma_start(out=own_sb[:],
                              in_=own_d.rearrange("(n p) d -> p n d", p=128)[:])
            c_sb = cst.tile([128, cfg.NW, cfg.G], BF16)
            nc.sync.dma_start(out=c_sb[:],
                              in_=c_d.rearrange("(n p) d -> p n d", p=128)[:])
            pool_ps = ops.tile([cfg.G, F], F32, tag="pool", name="pool_ps")
            for t in range(cfg.NW):
                nc.tensor.matmul(out=pool_ps[:], lhsT=c_sb[:, t, :],
                                 rhs=own_sb[:, t, :], start=(t == 0),
                                 stop=(t == cfg.NW - 1))
            pool_sb = io.tile([cfg.G, F], F32, tag="pool_sb")
            nc.vector.tensor_copy(out=pool_sb[:], in_=pool_ps[:])
            ptT_ps = ops.tile([F, cfg.G], F32, tag="ptT")
            nc.tensor.transpose(out=ptT_ps[:], in_=pool_sb[:],
                                identity=idf_sb[0:cfg.G, 0:cfg.G])
            ptT = io.tile([F, cfg.G], BF16, tag="ptTsb")
            nc.vector.tensor_copy(out=ptT[:], in_=ptT_ps[:])
            out_ps = ops.tile([cfg.G, cfg.CP], F32, tag="out")
            nc.tensor.matmul(out=out_ps[:], lhsT=ptT[:], rhs=w3_sb[:],
                             start=True, stop=False)
            nc.tensor.matmul(out=out_ps[:], lhsT=ng_sb[:], rhs=b3_sb[:],
                             start=False, stop=True)
            out_sb = io.tile([cfg.G, cfg.CP], F32, tag="out_sb")
            nc.vector.tensor_copy(out=out_sb[:], in_=out_ps[:])
            nc.sync.dma_start(out=out_d[:], in_=out_sb[:])
    nc.compile()
    return nc


def _bf16(a):
    return np.ascontiguousarray(np.asarray(a, dtype=np.float32)).astype(BF16NP)


def _tables(cfg, h):
    """4 contiguous quadrant tables (bf16, padded) + per-core own slabs."""
    out = []
    for i in range(4):
        t = np.zeros((cfg.QPAD, cfg.F), np.float32)
        t[:QN] = h[i * QN:(i + 1) * QN]
        out.append(_bf16(t))
    owns = []
    for c in range(NCORES):
        o = np.zeros((cfg.NPAD, cfg.F), np.float32)
        o[:cfg.NPC] = h[c * cfg.NPC:(c + 1) * cfg.NPC]
        owns.append(_bf16(o))
    return out, owns


def _layer_inputs(cfg, plan, tables, owns):
    maps = []
    ident = _bf16(np.eye(128, dtype=np.float32))
    for c in range(NCORES):
        m = {f"hq{i}": tables[i] for i in range(4)}
        m["own"] = owns[c]
        m["ident"] = ident
        m["eidx"] = plan.idx_wrapped(c)
        m["smat"] = plan.smat(c)
        maps.append(m)
    return maps


def _run(nc, in_maps):
    return run_bass_kernel_spmd(nc, in_maps, core_ids=list(range(NCORES))).results


def gin_forward(cfg, x, edge_index, edge_weight, batch,
                W1, b1, W2, b2, W3, b3, ncs=None):
    src = np.asarray(edge_index[0], np.int64)
    dst = np.asarray(edge_index[1], np.int64)
    plan = Plan(cfg, src, dst, np.asarray(edge_weight, np.float32))
    if ncs is None:
        ncs = {}
    if "l2" not in ncs:
        ncs["l2"] = build_layer(cfg, plan, cfg.F, True, False)
        ncs["l3"] = build_layer(cfg, plan, cfg.F, True, False)
        ncs["l4"] = build_pool(cfg)

    # layer 1: tables = x
    h = np.asarray(x, np.float32)
    tables, owns = _tables(cfg, h)
    maps = _layer_inputs(cfg, plan, tables, owns)
    for m in maps:
        m["W"] = _bf16(W1)
        m["bcol"] = np.asarray(b1, np.float32).reshape(128, 1)
    res = _run(ncs["l2"], maps)
    h1 = np.concatenate(
        [np.asarray(res[c]["hT"], np.float32).T[:cfg.NPC] for c in range(NCORES)])

    tables, owns = _tables(cfg, h1)
    maps = _layer_inputs(cfg, plan, tables, owns)
    for m in maps:
        m["W"] = _bf16(W2)
        m["bcol"] = np.asarray(b2, np.float32).reshape(128, 1)
    res = _run(ncs["l3"], maps)
    h2 = np.concatenate(
        [np.asarray(res[c]["hT"], np.float32).T[:cfg.NPC] for c in range(NCORES)])

    # layer 3 + pool: out = pool((I + A) h2) W3 + n_g b3. The pooled
    # (I + A) operator restricted to graph ids is a constant [N, G] matrix
    # C (edge structure only): C[j, g] = 1(batch_j = g) + sum_{e: src=j}
    # w_e 1(batch[dst_e] = g). Each core contracts its own h2 rows.
    W3p = np.zeros((cfg.F, cfg.CP), np.float32)
    W3p[:, :cfg.C] = np.asarray(W3, np.float32)
    b3p = np.zeros(cfg.CP, np.float32)
    b3p[:cfg.C] = np.asarray(b3, np.float32)
    batch64 = np.asarray(batch, np.int64)
    ng = np.bincount(batch64, minlength=cfg.G).astype(np.float32)
    Cm = np.zeros((cfg.N, cfg.G), np.float32)
    Cm[np.arange(cfg.N), batch64] = 1.0
    np.add.at(Cm, (src, batch64[dst]), np.asarray(edge_weight, np.float32))
    maps = []
    for c in range(NCORES):
        o = np.zeros((cfg.NPAD, cfg.F), np.float32)
        o[:cfg.NPC] = h2[c * cfg.NPC:(c + 1) * cfg.NPC]
        cp = np.zeros((cfg.NPAD, cfg.G), np.float32)
        cp[:cfg.NPC] = Cm[c * cfg.NPC:(c + 1) * cfg.NPC]
        maps.append({
            "own": _bf16(o), "Cmat": _bf16(cp),
            "identf": np.eye(128, dtype=np.float32),
            "W3p": _bf16(W3p),
            "ngrow": _bf16((ng if c == 0 else np.zeros_like(ng)).reshape(1, -1)),
            "b3row": _bf16(b3p.reshape(1, -1)),
        })
    res = _run(ncs["l4"], maps)
    out = np.zeros((cfg.G, cfg.CP), np.float32)
    for c in range(NCORES):
        out += np.asarray(res[c]["pool"], np.float32)
    return out[:, :cfg.C].astype(np.float32)


def kernel(x, edge_index, edge_weight, batch, W1, b1, W2, b2, W3, b3):
    cfg = Cfg()
    return gin_forward(cfg, x, edge_index, edge_weight, batch,
                       W1, b1, W2, b2, W3, b3)


# revision 5
# speedup vs baseline: 3.3260x; 1.4515x over previous
"""GIN (3-layer) message-passing kernel for 8 Trainium2 NeuronCores, v2.

Structure (3 device launches, one per GIN layer):
  layer(h) = relu((h + A h) @ W + b)   -- gather h rows directly (L2's table
  is the raw input x, so no separate dense-matmul launch), aggregate per dst
  window in PSUM, then apply W per window with W as the stationary matmul
  operand and bias fused into the ReLU on the scalar engine.

  Per core (1D dst-node partition, 12500 nodes = 98 windows of 128):
   - edge stream sorted by (window batch, src quadrant, window, src); per
     (window, quadrant) segment sized to the max count over cores so the
     SPMD instruction stream is shared; segments pack back-to-back (tiles
     that straddle a window boundary issue one S-matmul per window).
   - per 128-edge tile: gpsimd.dma_gather of h[src] rows (bf16, 256B) +
     one DVE-built S matrix (S[e,r] = w_e * (slot_e == r)) per covered
     window; TensorE accumulates either G^T S (layers 1-2, transposed
     aggregate so W stays stationary) or S^T G (layer 3).
   - self term: identity matmul with the core's own h tile per window.
   - layer 3 pools before the W3 projection (both linear): P = onehot^T agg
     accumulated across windows, then P @ W3pad + n_g b3 on device; host
     sums the 8 partial [G, CP] outputs.
"""

import numpy as np
import concourse.bass as bass
import concourse.mybir as mybir
import concourse.tile as tile
from concourse import bacc
from concourse.bass_utils import run_bass_kernel_spmd

try:
    import ml_dtypes
    BF16NP = ml_dtypes.bfloat16
except ImportError:  # pragma: no cover
    import jax.numpy as jnp
    BF16NP = jnp.bfloat16

F32 = mybir.dt.float32
BF16 = mybir.dt.bfloat16
I16 = mybir.dt.int16
AOT = mybir.AluOpType
ACT = mybir.ActivationFunctionType

NCORES = 8
WIN = 128
WB = 6             # windows per psum batch (1 PSUM bank each)
QN = 25000         # nodes per quadrant table
MAXCALL = 1024     # dma_gather HW limit per call
SCRATCH = 16384


class Cfg:
    def __init__(self, N=100000, E=1600000, F=128, C=40, G=64):
        self.N, self.E, self.F, self.C, self.G = N, E, F, C, G
        self.CP = 64
        self.NPC = N // NCORES
        self.NW = -(-self.NPC // WIN)
        self.NPAD = self.NW * WIN
        self.QPAD = -(-QN // 128) * 128


class Plan:
    """Shared-structure edge stream. All structure (segment offsets, tile
    window spans, call layout, stop flags) is identical across cores;
    only per-core data arrays (idx/slot/wgt) differ."""

    def __init__(self, cfg, src, dst, ew):
        self.cfg = cfg
        NPC, NW = cfg.NPC, cfg.NW
        core = dst // NPC
        dstl = dst - core * NPC
        w = dstl // WIN
        slot = dstl % WIN
        q = src // QN
        srcl = (src - q * QN).astype(np.int64)

        nb = -(-NW // WB)
        self.batches = [(b * WB, min((b + 1) * WB, NW)) for b in range(nb)]

        cnt = np.zeros((NCORES, NW, 4), np.int64)
        np.add.at(cnt, (core, w, q), 1)
        assert (cnt.max(axis=0) >= 1).all()
        # segment length = ceil(mean over cores); per-core overflow edges go
        # to a shared overflow zone at the end of each (batch, q) run.
        L = np.ceil(cnt.mean(axis=0)).astype(np.int64)   # [NW, 4]
        ov = np.maximum(cnt - L[None], 0)                # [C, NW, 4]

        seg_off = np.zeros((NW, 4), np.int64)    # absolute stream offset
        runbase = np.zeros((NW, 4), np.int64)    # overflow zone start of run
        self.runs = []                           # (b, q, off, padlen, seglen)
        cur = 0
        for b, (wlo, whi) in enumerate(self.batches):
            for qq in range(4):
                off = cur
                for ww in range(wlo, whi):
                    seg_off[ww, qq] = cur
                    cur += L[ww, qq]
                seglen = cur - off
                ov_max = int(ov[:, wlo:whi, qq].sum(axis=1).max())
                pad = -(-(seglen + ov_max) // 128) * 128
                runbase[wlo:whi, qq] = off + seglen
                self.runs.append((b, qq, off, pad, seglen))
                cur = off + pad
        self.NSLOT = cur
        self.NT = cur // 128

        # calls: chunks of <=MAXCALL within each run
        self.calls = []                          # (q, slot_off, nidx)
        self.batch_calls = [[] for _ in range(nb)]
        for (b, qq, off, pad, seglen) in self.runs:
            for rel in range(0, pad, MAXCALL):
                nidx = min(MAXCALL, pad - rel)
                self.batch_calls[b].append(len(self.calls))
                self.calls.append((qq, off + rel, nidx))
        # tile -> (call, k)
        self.tile_call = {}
        for ci, (qq, coff, nidx) in enumerate(self.calls):
            for k in range(nidx // 128):
                self.tile_call[coff // 128 + k] = (ci, k)

        # windows covered per tile: window segments, plus every batch
        # window for tiles overlapping the run's overflow zone.
        wsets = [set() for _ in range(self.NT)]
        for (b, qq, off, pad, seglen) in self.runs:
            wlo, whi = self.batches[b]
            for ww in range(wlo, whi):
                s0 = seg_off[ww, qq]
                s1 = s0 + L[ww, qq]
                for t in range(s0 // 128, -(-s1 // 128)):
                    wsets[t].add(ww)
            for t in range((off + seglen) // 128, (off + pad) // 128):
                wsets[t].update(range(wlo, whi))
        self.batch_tiles = []
        for b in range(nb):
            ts = set()
            for ci in self.batch_calls[b]:
                qq, coff, nidx = self.calls[ci]
                ts.update(range(coff // 128, (coff + nidx) // 128))
            self.batch_tiles.append(sorted(ts))
        # assign scol ids in execution order (batch-major, tile, window)
        # so each batch's S columns are one contiguous block in HBM.
        self.tile_w = [[] for _ in range(self.NT)]
        scol_of = {}
        self.batch_scol = []
        for b in range(nb):
            j_lo = len(scol_of)
            for t in self.batch_tiles[b]:
                for ww in sorted(wsets[t]):
                    scol_of[(t, ww)] = len(scol_of)
                    self.tile_w[t].append((ww, scol_of[(t, ww)]))
            self.batch_scol.append((j_lo, len(scol_of)))
        self.NS = len(scol_of)
        self.t_of_scol = np.zeros(self.NS, np.int64)
        for (t, ww), j in scol_of.items():
            self.t_of_scol[j] = t
        # stop flag: last (t, w) matmul per window in stream order
        last_mm = {}
        for t in range(self.NT):
            for (ww, j) in self.tile_w[t]:
                last_mm[ww] = (t, j)
        self.stop_at = {(t, j): ww for ww, (t, j) in last_mm.items()}

        # per-core data arrays
        order = np.lexsort((srcl, q, w, core))
        g_of_edge = ((core * NW + w) * 4 + q)
        sorted_g = g_of_edge[order]
        starts = np.searchsorted(sorted_g, np.arange(NCORES * NW * 4))
        rank = np.arange(len(order)) - starts[sorted_g]
        rank = rank[np.argsort(order, kind="stable")]
        # placement: first L edges of each (core, w, q) cell go to the
        # window segment; the rest go to the run's overflow zone, packed
        # in window order per core.
        opx = np.cumsum(ov, axis=1) - ov         # exclusive prefix within NW
        for b, (wlo, whi) in enumerate(self.batches):
            opx[:, wlo:whi, :] -= opx[:, wlo:wlo + 1, :]
        inseg = rank < L[w, q]
        pos = np.where(inseg, seg_off[w, q] + rank,
                       runbase[w, q] + opx[core, w, q] + (rank - L[w, q]))
        self.idx = np.zeros((NCORES, self.NSLOT), np.int16)
        self.wgt = np.zeros((NCORES, self.NSLOT), np.float32)
        self.slotv = np.full((NCORES, 128, self.NS), -1.0, np.float32)
        self.idx[core, pos] = srcl.astype(np.int16)
        self.wgt[core, pos] = ew.astype(np.float32)
        t_of = pos // 128
        p_of = pos % 128
        scol_arr = np.full((self.NT, NW), -1, np.int64)
        for (t, ww), j in scol_of.items():
            scol_arr[t, ww] = j
        j_of = scol_arr[t_of, w]
        assert (j_of >= 0).all()
        self.slotv[core, p_of, j_of] = slot.astype(np.float32)

    def idx_wrapped(self, c):
        a = self.idx[c].reshape(-1, 16).T
        return np.ascontiguousarray(np.tile(a, (8, 1)))

    def smat(self, c):
        """Precomputed S tiles [128, NS, 128] bf16: S[p, j, r] =
        wgt[t(j)*128+p] * (slotv[p, j] == r)."""
        if not hasattr(self, "_smat"):
            self._smat = {}
        if c not in self._smat:
            out = np.zeros((128, self.NS, 128), BF16NP)
            wt = self.wgt[c].reshape(self.NT, 128)
            r = np.arange(128)
            for j0 in range(0, self.NS, 256):
                j1 = min(j0 + 256, self.NS)
                sl = self.slotv[c][:, j0:j1]
                blk = (sl[:, :, None] == r).astype(np.float32)
                blk *= wt[self.t_of_scol[j0:j1]].T[:, :, None]
                out[:, j0:j1, :] = blk.astype(BF16NP)
            self._smat[c] = out
        return self._smat[c]


def _iota(n, m):
    return np.tile(np.arange(m, dtype=np.float32), (n, 1))


def build_layer(cfg, plan, FO, relu, pool):
    F = cfg.F
    nc = bacc.Bacc("TRN2", target_bir_lowering=False, debug=False,
                   num_devices=NCORES, dynamic_dma_scratch_size=2 * SCRATCH,
                   num_swdge_queues=2)
    hq = [nc.dram_tensor(f"hq{i}", [cfg.QPAD, F], BF16,
                         kind="ExternalInput").ap() for i in range(4)]
    own_d = nc.dram_tensor("own", [cfg.NPAD, F], BF16, kind="ExternalInput").ap()
    id_d = nc.dram_tensor("ident", [128, 128], BF16, kind="ExternalInput").ap()
    ix_d = nc.dram_tensor("eidx", [128, plan.NT * 8], I16,
                          kind="ExternalInput").ap()
    s_d = nc.dram_tensor("smat", [128, plan.NS, WIN], BF16,
                         kind="ExternalInput").ap()
    if pool:
        ig_d = nc.dram_tensor("iotaG", [128, cfg.G], F32,
                              kind="ExternalInput").ap()
        bv_d = nc.dram_tensor("bvals", [128, cfg.NW], F32,
                              kind="ExternalInput").ap()
        idf_d = nc.dram_tensor("identf", [128, 128], F32,
                               kind="ExternalInput").ap()
        w3_d = nc.dram_tensor("W3p", [F, cfg.CP], BF16,
                              kind="ExternalInput").ap()
        ng_d = nc.dram_tensor("ngrow", [1, cfg.G], BF16,
                              kind="ExternalInput").ap()
        b3_d = nc.dram_tensor("b3row", [1, cfg.CP], BF16,
                              kind="ExternalInput").ap()
        out_d = nc.dram_tensor("pool", [cfg.G, cfg.CP], F32,
                               kind="ExternalOutput").ap()
    else:
        w_d = nc.dram_tensor("W", [F, FO], BF16, kind="ExternalInput").ap()
        b_d = nc.dram_tensor("bcol", [128, 1], F32, kind="ExternalInput").ap()
        out_d = nc.dram_tensor("hT", [FO, cfg.NPAD], BF16,
                               kind="ExternalOutput").ap()

    with tile.TileContext(nc) as tc:
        with tc.tile_pool(name="cst", bufs=1) as cst, \
             tc.tile_pool(name="meta", bufs=1) as meta, \
             tc.tile_pool(name="gath", bufs=6) as gath, \
             tc.tile_pool(name="smat", bufs=2) as spool, \
             tc.tile_pool(name="sp", bufs=10) as sp, \
             tc.tile_pool(name="io", bufs=8) as io, \
             tc.tile_pool(name="aps", bufs=WB, space="PSUM") as aps, \
             tc.tile_pool(name="ops", bufs=1 if pool else 2,
                          space="PSUM") as ops:
            id_sb = cst.tile([128, 128], BF16)
            nc.sync.dma_start(out=id_sb[:], in_=id_d[:])
            if pool:
                ig_sb = cst.tile([128, cfg.G], F32)
                nc.sync.dma_start(out=ig_sb[:], in_=ig_d[:])
                bv_sb = cst.tile([128, cfg.NW], F32)
                nc.sync.dma_start(out=bv_sb[:], in_=bv_d[:])
                idf_sb = cst.tile([128, 128], F32)
                nc.sync.dma_start(out=idf_sb[:], in_=idf_d[:])
                w3_sb = cst.tile([F, cfg.CP], BF16)
                nc.sync.dma_start(out=w3_sb[:], in_=w3_d[:])
                ng_sb = cst.tile([1, cfg.G], BF16)
                nc.sync.dma_start(out=ng_sb[:], in_=ng_d[:])
                b3_sb = cst.tile([1, cfg.CP], BF16)
                nc.sync.dma_start(out=b3_sb[:], in_=b3_d[:])
                pool_ps = ops.tile([cfg.G, F], F32, tag="pool", name="pool_ps")
            else:
                w_sb = cst.tile([F, FO], BF16)
                nc.sync.dma_start(out=w_sb[:], in_=w_d[:])
                b_sb = cst.tile([128, 1], F32)
                nc.sync.dma_start(out=b_sb[:], in_=b_d[:])
            ix_sb = meta.tile([128, plan.NT * 8], I16)
            # batch 0's index columns first so the first gather starts early
            qq0, coff0, nidx0 = plan.calls[plan.batch_calls[0][-1]]
            c_split = (coff0 + nidx0) // 16
            nc.sync.dma_start(out=ix_sb[:, :c_split], in_=ix_d[:, :c_split])
            nc.sync.dma_start(out=ix_sb[:, c_split:], in_=ix_d[:, c_split:])
            own_sb = meta.tile([128, cfg.NW, F], BF16)
            own_r = own_d.rearrange("(n p) d -> p n d", p=128)
            nc.sync.dma_start(out=own_sb[:], in_=own_r[:])

            for b, (wlo, whi) in enumerate(plan.batches):
                tile_src = {}
                for ci in plan.batch_calls[b]:
                    qq, coff, nidx = plan.calls[ci]
                    ct = nidx // 128
                    gt = gath.tile([128, ct, F], BF16, tag="gt")
                    nc.gpsimd.dma_gather(
                        gt[:], hq[qq][:], ix_sb[:, coff // 16:(coff + nidx) // 16],
                        nidx, nidx, F, queue_num=ci % 2)
                    for k in range(ct):
                        tile_src[coff // 128 + k] = (gt, k)
                j_lo, j_hi = plan.batch_scol[b]
                sblk = spool.tile([128, j_hi - j_lo, WIN], BF16, tag="sblk")
                nc.sync.dma_start(out=sblk[:], in_=s_d[:, j_lo:j_hi, :])
                wps = {}
                for ww in range(wlo, whi):
                    pt = aps.tile([128, WIN], F32, tag="agg",
                                  name=f"agg_{b}_{ww}")
                    wps[ww] = pt[:]
                for ww in range(wlo, whi):
                    pw = wps[ww]
                    if pool:
                        nc.tensor.matmul(out=pw, lhsT=id_sb[:],
                                         rhs=own_sb[:, ww, :],
                                         start=True, stop=False)
                    else:
                        nc.tensor.matmul(out=pw, lhsT=own_sb[:, ww, :],
                                         rhs=id_sb[:], start=True, stop=False)
                for t in plan.batch_tiles[b]:
                    gt, k = tile_src[t]
                    for (ww, j) in plan.tile_w[t]:
                        s_t = sblk[:, j - j_lo, :]
                        is_stop = plan.stop_at.get((t, j)) == ww
                        if pool:
                            nc.tensor.matmul(out=wps[ww], lhsT=s_t,
                                             rhs=gt[:, k, :],
                                             start=False, stop=is_stop)
                        else:
                            nc.tensor.matmul(out=wps[ww], lhsT=gt[:, k, :],
                                             rhs=s_t,
                                             start=False, stop=is_stop)
                for ww in range(wlo, whi):
                    pw = wps[ww]
                    if pool:
                        agg_sb = io.tile([128, F], BF16, tag="agg_sb")
                        nc.vector.tensor_copy(out=agg_sb[:], in_=pw)
                        s_p = sp.tile([128, cfg.G], BF16, tag="SP")
                        nc.vector.tensor_scalar(
                            out=s_p[:], in0=ig_sb[:],
                            scalar1=bv_sb[:, ww:ww + 1], scalar2=None,
                            op0=AOT.is_equal)
                        nc.tensor.matmul(out=pool_ps[:], lhsT=s_p[:],
                                         rhs=agg_sb[:], start=(ww == 0),
                                         stop=(ww == cfg.NW - 1))
                    else:
                        aggT = io.tile([F, 128], BF16, tag="aggT")
                        nc.vector.tensor_copy(out=aggT[:], in_=pw)
                        preT = ops.tile([FO, 128], F32, tag="preT")
                        nc.tensor.matmul(out=preT[:], lhsT=w_sb[:],
                                         rhs=aggT[:], start=True, stop=True)
                        hT = io.tile([FO, 128], BF16, tag="hT")
                        nc.scalar.activation(
                            out=hT[:], in_=preT[:],
                            func=ACT.Relu if relu else ACT.Copy,
                            bias=b_sb[:])
                        nc.sync.dma_start(
                            out=out_d[:, ww * 128:(ww + 1) * 128], in_=hT[:])
            if pool:
                pool_sb = io.tile([cfg.G, F], F32, tag="pool_sb")
                nc.vector.tensor_copy(out=pool_sb[:], in_=pool_ps[:])
                ptT_ps = ops.tile([F, cfg.G], F32, tag="pool")
                nc.tensor.transpose(out=ptT_ps[:], in_=pool_sb[:],
                                    identity=idf_sb[0:cfg.G, 0:cfg.G])
                ptT = io.tile([F, cfg.G], BF16, tag="ptTsb")
                nc.vector.tensor_copy(out=ptT[:], in_=ptT_ps[:])
                out_ps = ops.tile([cfg.G, cfg.CP], F32, tag="pool")
                nc.tensor.matmul(out=out_ps[:], lhsT=ptT[:], rhs=w3_sb[:],
                                 start=True, stop=False)
                nc.tensor.matmul(out=out_ps[:], lhsT=ng_sb[:], rhs=b3_sb[:],
                                 start=False, stop=True)
                out_sb = io.tile([cfg.G, cfg.CP], F32, tag="out_sb")
                nc.vector.tensor_copy(out=out_sb[:], in_=out_ps[:])
                nc.sync.dma_start(out=out_d[:], in_=out_sb[:])
    nc.compile()
    return nc


def build_pool(cfg):
    """Final layer: P = C^T @ h2_own over the core's own rows (C is the
    host-precomputed pool-of-(I+A) operator restricted to this core's
    columns), then out = P @ W3pad + n_g b3. No gathers."""
    F = cfg.F
    nc = bacc.Bacc("TRN2", target_bir_lowering=False, debug=False,
                   num_devices=NCORES, dynamic_dma_scratch_size=SCRATCH)
    own_d = nc.dram_tensor("own", [cfg.NPAD, F], BF16, kind="ExternalInput").ap()
    c_d = nc.dram_tensor("Cmat", [cfg.NPAD, cfg.G], BF16,
                         kind="ExternalInput").ap()
    idf_d = nc.dram_tensor("identf", [128, 128], F32, kind="ExternalInput").ap()
    w3_d = nc.dram_tensor("W3p", [F, cfg.CP], BF16, kind="ExternalInput").ap()
    ng_d = nc.dram_tensor("ngrow", [1, cfg.G], BF16, kind="ExternalInput").ap()
    b3_d = nc.dram_tensor("b3row", [1, cfg.CP], BF16, kind="ExternalInput").ap()
    out_d = nc.dram_tensor("pool", [cfg.G, cfg.CP], F32,
                           kind="ExternalOutput").ap()
    with tile.TileContext(nc) as tc:
        with tc.tile_pool(name="cst", bufs=1) as cst, \
             tc.tile_pool(name="io", bufs=4) as io, \
             tc.tile_pool(name="ops", bufs=1, space="PSUM") as ops:
            idf_sb = cst.tile([128, 128], F32)
            nc.sync.dma_start(out=idf_sb[:], in_=idf_d[:])
            w3_sb = cst.tile([F, cfg.CP], BF16)
            nc.sync.dma_start(out=w3_sb[:], in_=w3_d[:])
            ng_sb = cst.tile([1, cfg.G], BF16)
            nc.sync.dma_start(out=ng_sb[:], in_=ng_d[:])
            b3_sb = cst.tile([1, cfg.CP], BF16)
            nc.sync.dma_start(out=b3_sb[:], in_=b3_d[:])
            own_sb = cst.tile([128, cfg.NW, F], BF16)
            nc.sync.dma_start(out=own_sb[:],
                              in_=own_d.rearrange("(n p) d -> p n d", p=128)[:])
            c_sb = cst.tile([128, cfg.NW, cfg.G], BF16)
            nc.sync.dma_start(out=c_sb[:],
                              in_=c_d.rearrange("(n p) d -> p n d", p=128)[:])
            pool_ps = ops.tile([cfg.G, F], F32, tag="pool", name="pool_ps")
            for t in range(cfg.NW):
                nc.tensor.matmul(out=pool_ps[:], lhsT=c_sb[:, t, :],
                                 rhs=own_sb[:, t, :], start=(t == 0),
                                 stop=(t == cfg.NW - 1))
            pool_sb = io.tile([cfg.G, F], F32, tag="pool_sb")
            nc.vector.tensor_copy(out=pool_sb[:], in_=pool_ps[:])
            ptT_ps = ops.tile([F, cfg.G], F32, tag="ptT")
            nc.tensor.transpose(out=ptT_ps[:], in_=pool_sb[:],
                                identity=idf_sb[0:cfg.G, 0:cfg.G])
            ptT = io.tile([F, cfg.G], BF16, tag="ptTsb")
            nc.vector.tensor_copy(out=ptT[:], in_=ptT_ps[:])
            out_ps = ops.tile([cfg.G, cfg.CP], F32, tag="out")
            nc.tensor.matmul(out=out_ps[:], lhsT=ptT[:], rhs=w3_sb[:],
                             start=True, stop=False)
            nc.tensor.matmul(out=out_ps[:], lhsT=ng_sb[:], rhs=b3_sb[:],
                             start=False, stop=True)
            out_sb = io.tile([cfg.G, cfg.CP], F32, tag="out_sb")
            nc.vector.tensor_copy(out=out_sb[:], in_=out_ps[:])
            nc.sync.dma_start(out=out_d[:], in_=out_sb[:])
    nc.compile()
    return nc


def _bf16(a):
    return np.ascontiguousarray(np.asarray(a, dtype=np.float32)).astype(BF16NP)


def _tables(cfg, h):
    """4 contiguous quadrant tables (bf16, padded) + per-core own slabs."""
    out = []
    for i in range(4):
        t = np.zeros((cfg.QPAD, cfg.F), np.float32)
        t[:QN] = h[i * QN:(i + 1) * QN]
        out.append(_bf16(t))
    owns = []
    for c in range(NCORES):
        o = np.zeros((cfg.NPAD, cfg.F), np.float32)
        o[:cfg.NPC] = h[c * cfg.NPC:(c + 1) * cfg.NPC]
        owns.append(_bf16(o))
    return out, owns


def _layer_inputs(cfg, plan, tables, owns):
    maps = []
    ident = _bf16(np.eye(128, dtype=np.float32))
    for c in range(NCORES):
        m = {f"hq{i}": tables[i] for i in range(4)}
        m["own"] = owns[c]
        m["ident"] = ident
        m["eidx"] = plan.idx_wrapped(c)
        m["smat"] = plan.smat(c)
        maps.append(m)
    return maps


def _run(nc, in_maps):
    return run_bass_kernel_spmd(nc, in_maps, core_ids=list(range(NCORES))).results


def gin_forward(cfg, x, edge_index, edge_weight, batch,
                W1, b1, W2, b2, W3, b3, ncs=None):
    src = np.asarray(edge_index[0], np.int64)
    dst = np.asarray(edge_index[1], np.int64)
    plan = Plan(cfg, src, dst, np.asarray(edge_weight, np.float32))
    if ncs is None:
        ncs = {}
    if "l2" not in ncs:
        ncs["l2"] = build_layer(cfg, plan, cfg.F, True, False)
        ncs["l3"] = build_layer(cfg, plan, cfg.F, True, False)
        ncs["l4"] = build_pool(cfg)

    # layer 1: tables = x
    h = np.asarray(x, np.float32)
    tables, owns = _tables(cfg, h)
    maps = _layer_inputs(cfg, plan, tables, owns)
    for m in maps:
        m["W"] = _bf16(W1)
        m["bcol"] = np.asarray(b1, np.float32).reshape(128, 1)
    res = _run(ncs["l2"], maps)
    h1 = np.concatenate(
        [np.asarray(res[c]["hT"], np.float32).T[:cfg.NPC] for c in range(NCORES)])

    tables, owns = _tables(cfg, h1)
    maps = _layer_inputs(cfg, plan, tables, owns)
    for m in maps:
        m["W"] = _bf16(W2)
        m["bcol"] = np.asarray(b2, np.float32).reshape(128, 1)
    res = _run(ncs["l3"], maps)
    h2 = np.concatenate(
        [np.asarray(res[c]["hT"], np.float32).T[:cfg.NPC] for c in range(NCORES)])

    # layer 3 + pool: out = pool((I + A) h2) W3 + n_g b3. The pooled
    # (I + A) operator restricted to graph ids is a constant [N, G] matrix
    # C (edge structure only): C[j, g] = 1(batch_j = g) + sum_{e: src=j}
    # w_e 1(batch[dst_e] = g). Each core contracts its own h2 rows.
    W3p = np.zeros((cfg.F, cfg.CP), np.float32)
    W3p[:, :cfg.C] = np.asarray(W3, np.float32)
    b3p = np.zeros(cfg.CP, np.float32)
    b3p[:cfg.C] = np.asarray(b3, np.float32)
    batch64 = np.asarray(batch, np.int64)
    ng = np.bincount(batch64, minlength=cfg.G).astype(np.float32)
    Cm = np.zeros((cfg.N, cfg.G), np.float32)
    Cm[np.arange(cfg.N), batch64] = 1.0
    np.add.at(Cm, (src, batch64[dst]), np.asarray(edge_weight, np.float32))
    maps = []
    for c in range(NCORES):
        o = np.zeros((cfg.NPAD, cfg.F), np.float32)
        o[:cfg.NPC] = h2[c * cfg.NPC:(c + 1) * cfg.NPC]
        cp = np.zeros((cfg.NPAD, cfg.G), np.float32)
        cp[:cfg.NPC] = Cm[c * cfg.NPC:(c + 1) * cfg.NPC]
        maps.append({
            "own": _bf16(o), "Cmat": _bf16(cp),
            "identf": np.eye(128, dtype=np.float32),
            "W3p": _bf16(W3p),
            "ngrow": _bf16((ng if c == 0 else np.zeros_like(ng)).reshape(1, -1)),
            "b3row": _bf16(b3p.reshape(1, -1)),
        })
    res = _run(ncs["l4"], maps)
    out = np.zeros((cfg.G, cfg.CP), np.float32)
    for c in range(NCORES):
        out += np.asarray(res[c]["pool"], np.float32)
    return out[:, :cfg.C].astype(np.float32)


def kernel(x, edge_index, edge_weight, batch, W1, b1, W2, b2, W3, b3):
    cfg = Cfg()
    return gin_forward(cfg, x, edge_index, edge_weight, batch,
                       W1, b1, W2, b2, W3, b3)
